# revision 6
# baseline (speedup 1.0000x reference)
"""RBF kernel layer (retrieval_knn): out = exp(-||x - p||^2) for x [131072, 64]
against 512 prototypes, distributed data-parallel over 8 NeuronCores.

Math: exp(-(x_sq + p_sq - 2*cross)) computed as exp(2*S + bias) where
  S[n, m]  = cross[n, m] - p_sq[m]/2   (one K=65 matmul: augmented ones-row in
                                        lhsT pairs with a -p_sq/2 row in rhs)
  bias[n]  = -x_sq[n]                  (per-partition bias of the ACT exp)

Per 128-row tile: DMA x in -> DVE tensor_tensor_reduce (gives -x_sq) ->
PE transpose (x to feature-major lhsT) -> DVE copy PSUM->SBUF ->
PE matmul [65,128]x[65,512] -> ACT exp with bias -> DMA out.
"""

import numpy as np

# Problem constants (hardcoded per harness contract; kernel.py is self-contained)
N = 131072
D = 64
M = 512
GAMMA = 1.0
NCORES = 8
NSHARD = N // NCORES  # 16384
P = 128
NT = NSHARD // P  # 128 tiles per core
LHS_SLOTS = 4  # manual rotation slots for lhsT (ones row initialized once)

# Matmul dtype strategy: "f32" (native fp32, 4 cyc/row, full precision) or
# "f32r" (1 cyc/row at N>=256, tf32-ish mantissa).
MM_MODE = "f32"

_cache = {}


def _build_bass(nshard=NSHARD):
    import concourse.mybir as mybir
    import concourse.tile as tile
    from concourse import bacc
    from concourse.masks import make_identity

    f32 = mybir.dt.float32
    nt = nshard // P

    nc = bacc.Bacc(None, target_bir_lowering=False)
    x_d = nc.dram_tensor("x", [nshard, D], f32, kind="ExternalInput")
    pa_d = nc.dram_tensor("pa", [D + 1, M], f32, kind="ExternalInput")
    out_d = nc.dram_tensor("out", [nshard, M], f32, kind="ExternalOutput")

    with tile.TileContext(nc) as tc:
        with (
            tc.tile_pool(name="singles", bufs=1) as singles,
            tc.tile_pool(name="xin", bufs=4) as xin,
            tc.tile_pool(name="scratch", bufs=2) as scratch,
            tc.tile_pool(name="bias", bufs=4) as biasp,
            tc.tile_pool(name="outp", bufs=4) as outp,
            tc.tile_pool(name="ps_t", bufs=3, space="PSUM") as ps_t,
            tc.tile_pool(name="ps_o", bufs=4, space="PSUM") as ps_o,
        ):
            # Replicated prototype table [65, 512]: rows 0..63 = protos.T,
            # row 64 = -p_sq/2
            pa_sb = singles.tile([D + 1, M], f32)
            nc.sync.dma_start(pa_sb[:], pa_d[:])

            ident = singles.tile([P, P], f32)
            make_identity(nc, ident[:])

            # lhsT slots, manually rotated so the ones-row (row 64) is
            # initialized once per slot instead of once per tile.
            lhs_slots = []
            for j in range(LHS_SLOTS):
                lhsT_sb = singles.tile([D + 1, P], f32, name=f"lhsT{j}")
                nc.vector.memset(lhsT_sb[D : D + 1, :], 1.0)
                lhs_slots.append(lhsT_sb)

            for i in range(nt):
                x_sb = xin.tile([P, D], f32)
                nc.sync.dma_start(x_sb[:], x_d[i * P : (i + 1) * P, :])

                # negxsq = -sum(x*x) along features
                sq = scratch.tile([P, D], f32, tag="sq")
                negxsq = biasp.tile([P, 1], f32, tag="negxsq")
                nc.vector.tensor_mul(sq[:], x_sb[:], x_sb[:])
                nc.vector.tensor_reduce(
                    negxsq[:],
                    sq[:],
                    axis=mybir.AxisListType.X,
                    op=mybir.AluOpType.add,
                    negate=True,
                )

                # Transpose x tile to feature-major [64, 128]
                xt_ps = ps_t.tile([D, P], f32, tag="xt")
                nc.tensor.transpose(xt_ps[:], x_sb[:], ident[:])

                lhsT = lhs_slots[i % LHS_SLOTS]
                nc.vector.tensor_copy(lhsT[0:D, :], xt_ps[:])

                # S = lhsT.T @ pa : [128, 512]
                psum = ps_o.tile([P, M], f32, tag="psum")
                if MM_MODE == "f32r":
                    f32r = mybir.dt.float32r
                    nc.tensor.matmul(
                        psum[:],
                        lhsT[:].bitcast(f32r),
                        pa_sb[:].bitcast(f32r),
                        start=True,
                        stop=True,
                    )
                else:
                    nc.tensor.matmul(
                        psum[:], lhsT[:], pa_sb[:], start=True, stop=True
                    )

                # out = exp(2*S - x_sq)
                o_sb = outp.tile([P, M], f32, tag="o")
                nc.scalar.activation(
                    o_sb[:],
                    psum[:],
                    mybir.ActivationFunctionType.Exp,
                    bias=negxsq[:],
                    scale=2.0,
                )

                nc.sync.dma_start(out_d[i * P : (i + 1) * P, :], o_sb[:])

    nc.finalize()
    return nc


def _get_nc():
    if "nc" not in _cache:
        _cache["nc"] = _build_bass()
    return _cache["nc"]


def _prep_inputs(x, prototypes):
    x = np.ascontiguousarray(np.asarray(x, dtype=np.float32))
    prototypes = np.ascontiguousarray(np.asarray(prototypes, dtype=np.float32))
    p_sq = np.sum(prototypes.astype(np.float64) ** 2, axis=1)
    pa = np.empty((D + 1, M), dtype=np.float32)
    pa[:D, :] = prototypes.T
    pa[D, :] = (-0.5 * p_sq).astype(np.float32)
    x_shards = np.split(x, NCORES, axis=0)
    in_maps = [{"x": np.ascontiguousarray(s), "pa": pa} for s in x_shards]
    return in_maps


def _run(inputs, trace=False):
    from concourse.bass_utils import run_bass_kernel_spmd

    in_maps = _prep_inputs(inputs["x"], inputs["prototypes"])
    nc = _get_nc()
    res = run_bass_kernel_spmd(
        nc, in_maps, core_ids=list(range(NCORES)), trace=trace
    )
    out = np.concatenate([r["out"] for r in res.results], axis=0)
    return out, res


def kernel(**inputs) -> np.ndarray:
    out, _ = _run(inputs, trace=False)
    return out
